# revision 3
# baseline (speedup 1.0000x reference)
"""DynamicConv (attention-over-kernel-bank conv2d) on 8 Trainium2 NeuronCores.

Data-parallel over batch N=32: 4 samples per core. Per core:
  1. pooled mean + tiny MLP + softmax(tau) -> pi [4 samples, 4 mixtures]
  2. per-sample kernel aggregation  aggT[ci, kh, kw, co] = sum_m pi[m] * Wbank
     (DVE scalar_tensor_tensor chain, fp32 accum, bf16 result)
  3. conv2d 3x3 pad 1 as 36 shifted matmuls accumulated in PSUM per
     [co_tile=128 x 512] output block (x padded to 66x66 on host, bf16)
  4. epilogue: + pi @ Bbank.T bias via ScalarE, DMA out fp32.
"""

from contextlib import ExitStack

import ml_dtypes
import numpy as np

import concourse.bass as bass
import concourse.tile as tile
from concourse import bacc, bass_utils, mybir

N, CI, CO, KK, H, W, M = 32, 256, 256, 3, 64, 64, 4
HID = CI // M
TAU = 1.0 / 30.0
NCORES = 8
NL = N // NCORES          # samples per core
CIT, COT = CI // 128, CO // 128
HP = H + 2                # padded spatial
CHUNK_ROWS = 8            # output rows per PSUM block (8*64 = 512 free)
CHUNKS = H // CHUNK_ROWS
TAPS = KK * KK

F32 = mybir.dt.float32
BF16 = mybir.dt.bfloat16
BF16_NP = ml_dtypes.bfloat16

_CACHE: dict = {}


def _emit(ctx: ExitStack, tc: tile.TileContext):
    nc = tc.nc
    AF = mybir.ActivationFunctionType
    ALU = mybir.AluOpType
    AX = mybir.AxisListType

    xpad_d = nc.dram_tensor("xpad", (NL, CIT, 128, HP, HP), BF16, kind="ExternalInput").ap()
    wb_d = nc.dram_tensor("wb", (M, CIT, 128, TAPS, CO), BF16, kind="ExternalInput").ap()
    w1t_d = nc.dram_tensor("w1t", (CIT, 128, HID), F32, kind="ExternalInput").ap()
    b1_d = nc.dram_tensor("b1c", (HID, 1), F32, kind="ExternalInput").ap()
    w2t_d = nc.dram_tensor("w2t", (HID, M), F32, kind="ExternalInput").ap()
    b2t_d = nc.dram_tensor("b2t", (NL, M), F32, kind="ExternalInput").ap()
    bbt_d = nc.dram_tensor("bbt", (COT, 128, M), F32, kind="ExternalInput").ap()
    y_d = nc.dram_tensor("y", (NL, COT, 128, CHUNKS, CHUNK_ROWS * W), F32, kind="ExternalOutput").ap()

    consts = ctx.enter_context(tc.tile_pool(name="consts", bufs=1))
    xp_pool = ctx.enter_context(tc.tile_pool(name="xp", bufs=1))
    aggf_pool = ctx.enter_context(tc.tile_pool(name="aggf", bufs=2))
    aggb_pool = ctx.enter_context(tc.tile_pool(name="aggb", bufs=2))
    outp = ctx.enter_context(tc.tile_pool(name="outp", bufs=4))
    cpsum = ctx.enter_context(tc.tile_pool(name="cpsum", bufs=4, space="PSUM"))
    mpsum = ctx.enter_context(tc.tile_pool(name="mpsum", bufs=1, space="PSUM"))

    # ---- resident constants ----
    wb_sb = consts.tile([128, M, CIT, TAPS, CO], BF16)
    for m in range(M):
        for t in range(CIT):
            nc.sync.dma_start(wb_sb[:, m, t], wb_d[m, t])
    w1t_sb = consts.tile([128, CIT, HID], F32)
    for t in range(CIT):
        nc.sync.dma_start(w1t_sb[:, t], w1t_d[t])
    b1_sb = consts.tile([HID, 1], F32)
    nc.sync.dma_start(b1_sb[:], b1_d[:])
    w2t_sb = consts.tile([HID, M], F32)
    nc.sync.dma_start(w2t_sb[:], w2t_d[:])
    b2t_sb = consts.tile([NL, M], F32)
    nc.sync.dma_start(b2t_sb[:], b2t_d[:])
    bbt_sb = consts.tile([128, COT, M], F32)
    for t in range(COT):
        nc.sync.dma_start(bbt_sb[:, t], bbt_d[t])

    # ---- x, padded bf16, all samples resident ----
    xp_sb = xp_pool.tile([128, NL, CIT, HP, HP], BF16)
    for n in range(NL):
        for t in range(CIT):
            nc.sync.dma_start(xp_sb[:, n, t], xpad_d[n, t])

    # ---- global average pool (sum; 1/(H*W) folded into w1t host-side) ----
    pooled = consts.tile([128, CIT, NL], F32)
    for n in range(NL):
        for t in range(CIT):
            nc.vector.reduce_sum(pooled[:, t, n : n + 1], xp_sb[:, n, t], axis=AX.XY)

    # ---- attention MLP (batched over the 4 samples) ----
    hmid_ps = mpsum.tile([HID, NL], F32)
    for t in range(CIT):
        nc.tensor.matmul(hmid_ps[:], w1t_sb[:, t], pooled[:, t], start=(t == 0), stop=(t == CIT - 1))
    hmid_sb = consts.tile([HID, NL], F32)
    nc.scalar.activation(hmid_sb[:], hmid_ps[:], AF.Relu, bias=b1_sb[:])

    logit_ps = mpsum.tile([NL, M], F32)
    nc.tensor.matmul(logit_ps[:], hmid_sb[:], w2t_sb[:], start=True, stop=True)
    lt = consts.tile([NL, M], F32)
    nc.vector.scalar_tensor_tensor(lt[:], logit_ps[:], TAU, b2t_sb[:], op0=ALU.mult, op1=ALU.add)
    negmax = consts.tile([NL, 1], F32)
    nc.vector.reduce_max(negmax[:], lt[:], axis=AX.X, negate=True)
    pexp = consts.tile([NL, M], F32)
    nc.scalar.activation(pexp[:], lt[:], AF.Exp, bias=negmax[:])
    ssum = consts.tile([NL, 1], F32)
    nc.vector.reduce_sum(ssum[:], pexp[:], axis=AX.X)
    rsum = consts.tile([NL, 1], F32)
    nc.vector.reciprocal(rsum[:], ssum[:])
    pi_sb = consts.tile([NL, M], F32)
    nc.vector.tensor_scalar_mul(pi_sb[:], pexp[:], rsum[:])

    # pi broadcast across partitions: pi_b[:, n*M+m] = pi[n, m].
    # partition_broadcast needs a partition-0 source, so first collapse the
    # [NL, M] tile onto one partition with a tiny SBUF->SBUF DMA.
    pi_row = consts.tile([1, NL * M], F32)
    nc.sync.dma_start(pi_row[0:1, :].rearrange("p (n m) -> p n m", n=NL), pi_sb[:, :])
    pi_b = consts.tile([128, NL * M], F32)
    nc.gpsimd.partition_broadcast(pi_b[:, :], pi_row[0:1, :])

    # per-(co_tile, sample) bias column: bnT[co, n] = sum_m Bbank[co, m] * pi[n, m]
    bnT = consts.tile([128, COT, NL], F32)
    prod = consts.tile([128, M], F32)
    for ct in range(COT):
        for n in range(NL):
            nc.vector.tensor_mul(prod[:], bbt_sb[:, ct], pi_b[:, n * M : (n + 1) * M])
            nc.vector.reduce_sum(bnT[:, ct, n : n + 1], prod[:], axis=AX.X)

    # ---- per-sample: aggregate kernel, conv sweep ----
    for n in range(NL):
        acc = aggf_pool.tile([128, CIT, TAPS, CO], F32, tag="acc")
        agg = aggb_pool.tile([128, CIT, TAPS, CO], BF16, tag="agg")
        s = n * M
        for t in range(CIT):
            nc.vector.tensor_scalar_mul(acc[:, t], wb_sb[:, 0, t], pi_b[:, s : s + 1])
            nc.vector.scalar_tensor_tensor(acc[:, t], wb_sb[:, 1, t], pi_b[:, s + 1 : s + 2], acc[:, t], op0=ALU.mult, op1=ALU.add)
            nc.vector.scalar_tensor_tensor(acc[:, t], wb_sb[:, 2, t], pi_b[:, s + 2 : s + 3], acc[:, t], op0=ALU.mult, op1=ALU.add)
            nc.vector.scalar_tensor_tensor(agg[:, t], wb_sb[:, 3, t], pi_b[:, s + 3 : s + 4], acc[:, t], op0=ALU.mult, op1=ALU.add)

        for ct in range(COT):
            for c in range(CHUNKS):
                ps = cpsum.tile([128, CHUNK_ROWS * W], F32, tag="ps")
                i = 0
                for t in range(CIT):
                    for kh in range(KK):
                        for kw in range(KK):
                            nc.tensor.matmul(
                                ps[:],
                                agg[:, t, kh * KK + kw, ct * 128 : (ct + 1) * 128],
                                xp_sb[:, n, t, c * CHUNK_ROWS + kh : c * CHUNK_ROWS + kh + CHUNK_ROWS, kw : kw + W],
                                start=(i == 0),
                                stop=(i == CIT * TAPS - 1),
                            )
                            i += 1
                ot = outp.tile([128, CHUNK_ROWS * W], F32, tag="ot")
                nc.scalar.activation(ot[:], ps[:], AF.Identity, bias=bnT[:, ct, n : n + 1])
                nc.sync.dma_start(y_d[n, ct, :, c], ot[:])


def build_program():
    nc = bacc.Bacc("TRN2", target_bir_lowering=False, debug=False, num_devices=NCORES)
    with tile.TileContext(nc) as tc:
        with ExitStack() as ctx:
            _emit(ctx, tc)
    nc.compile()
    return nc


def prep_inputs(x, Wbank, Bbank, w1, b1, w2, b2):
    """Host-side layout prep. Returns per-core in_maps."""
    x4 = x.reshape(N, CIT, 128, H, W)
    xpad = np.zeros((N, CIT, 128, HP, HP), dtype=BF16_NP)
    xpad[:, :, :, 1 : H + 1, 1 : W + 1] = x4
    wb = np.ascontiguousarray(Wbank.transpose(1, 2, 3, 4, 0)).reshape(M, CIT, 128, TAPS, CO).astype(BF16_NP)
    w1t = np.ascontiguousarray((np.asarray(w1) / float(H * W)).T).reshape(CIT, 128, HID).astype(np.float32)
    b1c = np.asarray(b1, dtype=np.float32).reshape(HID, 1)
    w2t = np.ascontiguousarray(np.asarray(w2).T).astype(np.float32)
    b2t = np.tile(np.asarray(b2, dtype=np.float32) * TAU, (NL, 1))
    bbt = np.asarray(Bbank, dtype=np.float32).reshape(COT, 128, M)
    shared = {"wb": wb, "w1t": w1t, "b1c": b1c, "w2t": w2t, "b2t": b2t, "bbt": bbt}
    return [{"xpad": np.ascontiguousarray(xpad[c * NL : (c + 1) * NL]), **shared} for c in range(NCORES)]


def kernel(x, Wbank, Bbank, w1, b1, w2, b2):
    x = np.asarray(x, dtype=np.float32)
    in_maps = prep_inputs(x, Wbank, Bbank, w1, b1, w2, b2)
    if "nc" not in _CACHE:
        _CACHE["nc"] = build_program()
    res = bass_utils.run_bass_kernel_spmd(_CACHE["nc"], in_maps, core_ids=list(range(NCORES)))
    return np.concatenate([r["y"].reshape(NL, CO, H, W) for r in res.results], axis=0)


# revision 6
# speedup vs baseline: 1.0725x; 1.0725x over previous
"""DynamicConv (attention-over-kernel-bank conv2d) on 8 Trainium2 NeuronCores.

Data-parallel over batch N=32: 4 samples per core. Per core:
  1. pooled mean + tiny MLP + softmax(tau) -> pi [4 samples, 4 mixtures]
  2. per-sample kernel aggregation  aggT[ci, kh, kw, co] = sum_m pi[m] * Wbank
     (DVE scalar_tensor_tensor chain, fp32 accum, bf16 result)
  3. conv2d 3x3 pad 1 as 36 shifted matmuls accumulated in PSUM per
     [co_tile=128 x 512] output block (x padded to 66x66 on host, bf16)
  4. epilogue: + pi @ Bbank.T bias via ScalarE, DMA out fp32.
"""

from contextlib import ExitStack

import ml_dtypes
import numpy as np

import concourse.bass as bass
import concourse.tile as tile
from concourse import bacc, bass_utils, mybir

N, CI, CO, KK, H, W, M = 32, 256, 256, 3, 64, 64, 4
HID = CI // M
TAU = 1.0 / 30.0
NCORES = 8
NL = N // NCORES          # samples per core
CIT, COT = CI // 128, CO // 128
HP = H + 2                # padded spatial
CHUNK_ROWS = 8            # output rows per PSUM block (8*64 = 512 free)
CHUNKS = H // CHUNK_ROWS
TAPS = KK * KK

F32 = mybir.dt.float32
BF16 = mybir.dt.bfloat16
BF16_NP = ml_dtypes.bfloat16

_CACHE: dict = {}


def _emit(ctx: ExitStack, tc: tile.TileContext):
    nc = tc.nc
    AF = mybir.ActivationFunctionType
    ALU = mybir.AluOpType
    AX = mybir.AxisListType

    xpad_d = nc.dram_tensor("xpad", (NL, CIT, 128, HP, HP), BF16, kind="ExternalInput").ap()
    wb_d = nc.dram_tensor("wb", (M, CIT, 128, TAPS, CO), BF16, kind="ExternalInput").ap()
    w1t_d = nc.dram_tensor("w1t", (CIT, 128, HID), F32, kind="ExternalInput").ap()
    b1_d = nc.dram_tensor("b1c", (HID, 1), F32, kind="ExternalInput").ap()
    w2t_d = nc.dram_tensor("w2t", (HID, M), F32, kind="ExternalInput").ap()
    b2t_d = nc.dram_tensor("b2t", (NL, M), F32, kind="ExternalInput").ap()
    bbt_d = nc.dram_tensor("bbt", (COT, 128, M), F32, kind="ExternalInput").ap()
    y_d = nc.dram_tensor("y", (NL, COT, 128, CHUNKS, CHUNK_ROWS * W), F32, kind="ExternalOutput").ap()

    consts = ctx.enter_context(tc.tile_pool(name="consts", bufs=1))
    xp_pool = ctx.enter_context(tc.tile_pool(name="xp", bufs=1))
    aggb_pool = ctx.enter_context(tc.tile_pool(name="aggb", bufs=2))
    outp = ctx.enter_context(tc.tile_pool(name="outp", bufs=8))
    cpsum = ctx.enter_context(tc.tile_pool(name="cpsum", bufs=6, space="PSUM"))
    mpsum = ctx.enter_context(tc.tile_pool(name="mpsum", bufs=1, space="PSUM"))

    # ---- x, padded bf16, all samples resident (loaded first: pooling +
    # the batched MLP gate everything downstream) ----
    xp_sb = xp_pool.tile([128, NL, CIT, HP, HP], BF16)
    for n in range(NL):
        for t in range(CIT):
            nc.sync.dma_start(xp_sb[:, n, t], xpad_d[n, t])

    # ---- resident constants ----
    wb_sb = consts.tile([128, M, CIT, TAPS, CO], BF16)
    for m in range(M):
        for t in range(CIT):
            nc.sync.dma_start(wb_sb[:, m, t], wb_d[m, t])
    w1t_sb = consts.tile([128, CIT, HID], F32)
    for t in range(CIT):
        nc.sync.dma_start(w1t_sb[:, t], w1t_d[t])
    b1_sb = consts.tile([HID, 1], F32)
    nc.sync.dma_start(b1_sb[:], b1_d[:])
    w2t_sb = consts.tile([HID, M], F32)
    nc.sync.dma_start(w2t_sb[:], w2t_d[:])
    b2t_sb = consts.tile([NL, M], F32)
    nc.sync.dma_start(b2t_sb[:], b2t_d[:])
    bbt_sb = consts.tile([128, COT, M], F32)
    for t in range(COT):
        nc.sync.dma_start(bbt_sb[:, t], bbt_d[t])

    # ---- global average pool (sum; 1/(H*W) folded into w1t host-side) ----
    pooled = consts.tile([128, CIT, NL], F32)
    for n in range(NL):
        for t in range(CIT):
            nc.vector.reduce_sum(pooled[:, t, n : n + 1], xp_sb[:, n, t], axis=AX.XY)

    # ---- attention MLP (batched over the 4 samples) ----
    hmid_ps = mpsum.tile([HID, NL], F32)
    for t in range(CIT):
        nc.tensor.matmul(hmid_ps[:], w1t_sb[:, t], pooled[:, t], start=(t == 0), stop=(t == CIT - 1))
    hmid_sb = consts.tile([HID, NL], F32)
    nc.scalar.activation(hmid_sb[:], hmid_ps[:], AF.Relu, bias=b1_sb[:])

    logit_ps = mpsum.tile([NL, M], F32)
    nc.tensor.matmul(logit_ps[:], hmid_sb[:], w2t_sb[:], start=True, stop=True)
    lt = consts.tile([NL, M], F32)
    nc.vector.scalar_tensor_tensor(lt[:], logit_ps[:], TAU, b2t_sb[:], op0=ALU.mult, op1=ALU.add)
    negmax = consts.tile([NL, 1], F32)
    nc.vector.reduce_max(negmax[:], lt[:], axis=AX.X, negate=True)
    pexp = consts.tile([NL, M], F32)
    nc.scalar.activation(pexp[:], lt[:], AF.Exp, bias=negmax[:])
    ssum = consts.tile([NL, 1], F32)
    nc.vector.reduce_sum(ssum[:], pexp[:], axis=AX.X)
    rsum = consts.tile([NL, 1], F32)
    nc.vector.reciprocal(rsum[:], ssum[:])
    pi_sb = consts.tile([NL, M], F32)
    nc.vector.tensor_scalar_mul(pi_sb[:], pexp[:], rsum[:])

    # pi broadcast across partitions: pi_b[:, n*M+m] = pi[n, m].
    # partition_broadcast needs a partition-0 source, so first collapse the
    # [NL, M] tile onto one partition with a tiny SBUF->SBUF DMA.
    pi_row = consts.tile([1, NL * M], F32)
    nc.sync.dma_start(pi_row[0:1, :].rearrange("p (n m) -> p n m", n=NL), pi_sb[:, :])
    pi_b = consts.tile([128, NL * M], F32)
    nc.gpsimd.partition_broadcast(pi_b[:, :], pi_row[0:1, :])

    # per-(co_tile, sample) bias column: bnT[co, n] = sum_m Bbank[co, m] * pi[n, m]
    bnT = consts.tile([128, COT, NL], F32)
    prod = consts.tile([128, M], F32)
    for ct in range(COT):
        for n in range(NL):
            nc.vector.tensor_mul(prod[:], bbt_sb[:, ct], pi_b[:, n * M : (n + 1) * M])
            nc.vector.reduce_sum(bnT[:, ct, n : n + 1], prod[:], axis=AX.X)

    # ---- per-sample: aggregate kernel, conv sweep ----
    for n in range(NL):
        acc = aggb_pool.tile([128, CIT, TAPS, CO], BF16, tag="acc")
        agg = aggb_pool.tile([128, CIT, TAPS, CO], BF16, tag="agg")
        s = n * M
        for t in range(CIT):
            nc.vector.tensor_scalar_mul(acc[:, t], wb_sb[:, 0, t], pi_b[:, s : s + 1])
            nc.vector.scalar_tensor_tensor(acc[:, t], wb_sb[:, 1, t], pi_b[:, s + 1 : s + 2], acc[:, t], op0=ALU.mult, op1=ALU.add)
            nc.vector.scalar_tensor_tensor(acc[:, t], wb_sb[:, 2, t], pi_b[:, s + 2 : s + 3], acc[:, t], op0=ALU.mult, op1=ALU.add)
            nc.vector.scalar_tensor_tensor(agg[:, t], wb_sb[:, 3, t], pi_b[:, s + 3 : s + 4], acc[:, t], op0=ALU.mult, op1=ALU.add)

        for ct in range(COT):
            for c in range(CHUNKS):
                ps = cpsum.tile([128, CHUNK_ROWS * W], F32, tag="ps")
                i = 0
                for t in range(CIT):
                    for kh in range(KK):
                        for kw in range(KK):
                            nc.tensor.matmul(
                                ps[:],
                                agg[:, t, kh * KK + kw, ct * 128 : (ct + 1) * 128],
                                xp_sb[:, n, t, c * CHUNK_ROWS + kh : c * CHUNK_ROWS + kh + CHUNK_ROWS, kw : kw + W],
                                start=(i == 0),
                                stop=(i == CIT * TAPS - 1),
                            )
                            i += 1
                ot = outp.tile([128, CHUNK_ROWS * W], F32, tag="ot")
                nc.vector.tensor_scalar_add(ot[:], ps[:], bnT[:, ct, n : n + 1])
                nc.sync.dma_start(y_d[n, ct, :, c], ot[:])


def build_program():
    nc = bacc.Bacc("TRN2", target_bir_lowering=False, debug=False, num_devices=NCORES)
    with tile.TileContext(nc) as tc:
        with ExitStack() as ctx:
            _emit(ctx, tc)
    nc.compile()
    return nc


def prep_inputs(x, Wbank, Bbank, w1, b1, w2, b2):
    """Host-side layout prep. Returns per-core in_maps."""
    x4 = x.reshape(N, CIT, 128, H, W)
    xpad = np.zeros((N, CIT, 128, HP, HP), dtype=BF16_NP)
    xpad[:, :, :, 1 : H + 1, 1 : W + 1] = x4
    wb = np.ascontiguousarray(Wbank.transpose(1, 2, 3, 4, 0)).reshape(M, CIT, 128, TAPS, CO).astype(BF16_NP)
    w1t = np.ascontiguousarray((np.asarray(w1) / float(H * W)).T).reshape(CIT, 128, HID).astype(np.float32)
    b1c = np.asarray(b1, dtype=np.float32).reshape(HID, 1)
    w2t = np.ascontiguousarray(np.asarray(w2).T).astype(np.float32)
    b2t = np.tile(np.asarray(b2, dtype=np.float32) * TAU, (NL, 1))
    bbt = np.asarray(Bbank, dtype=np.float32).reshape(COT, 128, M)
    shared = {"wb": wb, "w1t": w1t, "b1c": b1c, "w2t": w2t, "b2t": b2t, "bbt": bbt}
    return [{"xpad": np.ascontiguousarray(xpad[c * NL : (c + 1) * NL]), **shared} for c in range(NCORES)]


def kernel(x, Wbank, Bbank, w1, b1, w2, b2):
    x = np.asarray(x, dtype=np.float32)
    in_maps = prep_inputs(x, Wbank, Bbank, w1, b1, w2, b2)
    if "nc" not in _CACHE:
        _CACHE["nc"] = build_program()
    res = bass_utils.run_bass_kernel_spmd(_CACHE["nc"], in_maps, core_ids=list(range(NCORES)))
    return np.concatenate([r["y"].reshape(NL, CO, H, W) for r in res.results], axis=0)
